# revision 19
# baseline (speedup 1.0000x reference)
"""CML int8-sim Trainium2 kernel.

Strategy (data-parallel over batch, 8 cores, B=256 -> 32 rows/core):
The per-step update
    mapped = r*g*(1-g)
    coupled = 0.5*(circ_conv(mapped, K) + mapped @ W_cc)
    g' = (1-beta)*((1-eps)*mapped + eps*coupled) + beta*drive
    g  = round(127*g')/127
is linear in `mapped`, so conv + coupling + site scalings fold into one
matrix.  State is kept 127-scaled: v = 127*g is an EXACT INTEGER in
[0,127] after every round, hence u = v*(127-v) is an even integer
<= 4032 -- exactly representable in fp16.  Folding r into the rows of
the weight matrix and pulling the diagonal passthrough out onto the DVE
makes the whole step a SINGLE fp16 matmul:
    127*g' = u @ W5*2^14 * 2^-14 + dd*u + 127*beta*drive
    W5[n,m] = (r_n/127) * 0.5*(1-beta_m)*eps_m*(W_cc+C)[n,m]   (fp16, 2^14-scaled)
    dd = (1-beta)*(1-eps)*r/127                                 (DVE elementwise)
fp16 streams (vs fp32) also relieve the XBUS budget of the 4-way
col-tiled PE matmuls.  Step 0's state (127*drive) is continuous, so its
lhs uses an exact hi+lo fp16 split (2 matmul chains, one step only).

Layout: scrambled [128, 512] (partition = 32*jgrp + batch) so all
elementwise work uses the full 128 partitions; per-step tensors split
into column halves so the DVE/ACT tail of one half overlaps the PE
matmuls of the other (block-major k order).
"""
import numpy as np

B, N, NCORES = 256, 2048, 8
BL = B // NCORES          # 32 batch rows per core
MAGIC = 12582912.0        # 1.5*2^23: (x+M)-M == RNE round for |x| < 2^22
WSCALE = 16384.0          # 2^14: keeps fp16 W entries out of subnormal range
LOSC = 2048.0             # 2^11: scale for the step-0 lo lhs part

MODE = "h16"              # "h16" (fp16 W, default) | "fp32p" (legacy fp32)
_programs = {}
_last_in_maps = None


def _build_program(steps, mode="h16", wbufs=2):
    import concourse.mybir as mybir
    import concourse.tile as tile
    from concourse import bacc

    f32 = mybir.dt.float32
    f16 = mybir.dt.float16
    sub = mybir.AluOpType.subtract
    add = mybir.AluOpType.add
    mult = mybir.AluOpType.mult

    nc = bacc.Bacc("TRN2", target_bir_lowering=False, debug=False)

    wdt = f16 if mode == "h16" else f32
    d_Wp = nc.dram_tensor("Wp", [128, 16 * N], wdt, kind="ExternalInput").ap()
    d_g0 = nc.dram_tensor("g0_bn", [128, 512], f32, kind="ExternalInput").ap()
    d_C = nc.dram_tensor("C_bn", [128, 512], f32, kind="ExternalInput").ap()
    d_R1 = nc.dram_tensor("R1_bn", [128, 512], f32, kind="ExternalInput").ap()
    d_R2 = nc.dram_tensor("R2_bn", [128, 512], f32, kind="ExternalInput").ap()
    d_id = nc.dram_tensor("ident", [128, 128], f32, kind="ExternalInput").ap()
    d_out = nc.dram_tensor("v_out", [128, 512], f32, kind="ExternalOutput").ap()

    ORDER = [0, 4, 8, 12, 1, 5, 9, 13, 2, 6, 10, 14, 3, 7, 11, 15]

    with tile.TileContext(nc) as tc:
        with tc.tile_pool(name="consts", bufs=1) as cp, \
             tc.tile_pool(name="work", bufs=wbufs) as wp, \
             tc.tile_pool(name="psum", bufs=2, space="PSUM") as pp, \
             tc.tile_pool(name="psumt", bufs=1, space="PSUM") as ppt, \
             tc.tile_pool(name="psuml", bufs=1, space="PSUM") as ppl:
            s_Wp = cp.tile([128, 16 * N], wdt)
            s_g0 = cp.tile([128, 512], f32)
            s_C = cp.tile([128, 512], f32)
            s_R1 = cp.tile([128, 512], f32)
            s_R2 = cp.tile([128, 512], f32)
            s_id = cp.tile([128, 128], f32)
            nc.sync.dma_start(out=s_g0[:], in_=d_g0[:])
            nc.sync.dma_start(out=s_C[:], in_=d_C[:])
            nc.sync.dma_start(out=s_R1[:], in_=d_R1[:])
            nc.sync.dma_start(out=s_R2[:], in_=d_R2[:])
            nc.sync.dma_start(out=s_id[:], in_=d_id[:])
            # W is big: DMA per 128-chunk so first matmuls can start
            # before the whole matrix has landed.
            for kc in range(16):
                nc.sync.dma_start(out=s_Wp[:, N * kc:N * (kc + 1)],
                                  in_=d_Wp[:, N * kc:N * (kc + 1)])

            def lhs_slice(tiles, kc):
                bk = kc % 4
                off = 128 * (bk % 2) + 32 * (kc // 4)
                return tiles[bk // 2][:, off:off + 32]

            if mode == "h16":
                # R1_bn carries DD = -(1-beta)(1-eps)*r/127 broadcast; R2 unused
                s_DD = s_R1
                s_id16 = cp.tile([128, 128], f16)
                nc.vector.tensor_copy(out=s_id16[:], in_=s_id[:])
                vh_prev = None
                for t in range(steps):
                    mts, mlos, dt2s = [], [], []
                    # one bank-wide transpose target shared by both halves
                    if t == 0:
                        pTm = ppt.tile([128, 512], f32, tag="pT")
                    else:
                        pT16m = ppt.tile([128, 512], f16, tag="pT16")
                    for h in (0, 1):
                        hs = slice(256 * h, 256 * (h + 1))
                        if t == 0:
                            # continuous state: full-precision path + hi/lo
                            src = s_g0[:, hs]
                            e1 = wp.tile([128, 256], f32, tag=f"e1{h}")
                            nc.vector.scalar_tensor_tensor(
                                out=e1[:], in0=src, scalar=127.0, in1=src,
                                op0=sub, op1=mult)
                            dsrc = e1
                            for b in range(2):
                                nc.tensor.transpose(
                                    pTm[:, 256 * h + 128 * b:
                                        256 * h + 128 * (b + 1)],
                                    e1[:, 128 * b:128 * (b + 1)], s_id[:])
                            mt = wp.tile([128, 256], f16, tag=f"mT{h}")
                            nc.scalar.copy(mt[:], pTm[:, hs])
                            mts.append(mt)
                            # lo part: ml = (pT - mt) * 2^11, transposed space
                            rl = wp.tile([128, 256], f32, tag=f"rl{h}")
                            nc.vector.tensor_sub(
                                out=rl[:], in0=pTm[:, hs], in1=mt[:])
                            ml = wp.tile([128, 256], f16, tag=f"ml{h}")
                            nc.vector.tensor_scalar(
                                out=ml[:], in0=rl[:], scalar1=LOSC,
                                scalar2=None, op0=mult)
                            mlos.append(ml)
                        else:
                            # m16 = (v - 127) * v  == -u : even integer
                            # <= 4032, EXACT in fp16
                            src = vh_prev[h][:]
                            m16 = wp.tile([128, 256], f16, tag=f"m16{h}")
                            nc.vector.scalar_tensor_tensor(
                                out=m16[:], in0=src, scalar=127.0, in1=src,
                                op0=sub, op1=mult)
                            dsrc = m16
                            for b in range(2):
                                nc.tensor.transpose(
                                    pT16m[:, 256 * h + 128 * b:
                                          256 * h + 128 * (b + 1)],
                                    m16[:, 128 * b:128 * (b + 1)], s_id16[:])
                            # split copies so matmuls can start after block 0
                            mt = wp.tile([128, 256], f16, tag=f"mT{h}")
                            nc.scalar.copy(mt[:, 0:128],
                                           pT16m[:, 256 * h:256 * h + 128])
                            nc.scalar.copy(mt[:, 128:256],
                                           pT16m[:, 256 * h + 128:
                                                 256 * (h + 1)])
                            mts.append(mt)
                        # dt2 = dsrc*DD + C == dd*u + 127*beta*drive (hidden
                        # under the matmuls)
                        dt1 = wp.tile([128, 256], f32, tag=f"dt1{h}")
                        nc.vector.tensor_mul(
                            out=dt1[:], in0=dsrc[:],
                            in1=s_DD[:, 256 * h:256 * (h + 1)])
                        dt2 = wp.tile([128, 256], f32, tag=f"dt2{h}")
                        nc.vector.tensor_add(
                            out=dt2[:], in0=dt1[:],
                            in1=s_C[:, 256 * h:256 * (h + 1)])
                        dt2s.append(dt2)

                    # Separate PSUM bank per half so the two accumulation
                    # chains can interleave at bk-group granularity
                    # [h0|bk01, h1|bk01, h0|bk23, h1|bk23] without two open
                    # groups sharing a bank.  Each chain keeps its internal
                    # round order (numerics unchanged), but the rounds that
                    # need the other half's transposed tile (bk23 ->
                    # mts[1]) land a full group later, giving both
                    # post-round chains ~2 groups of slack before the next
                    # step's matmuls consume their output.
                    P1a = pp.tile([128, 256], f32, tag="P1a")
                    P1b = pp.tile([128, 256], f32, tag="P1b")
                    P1h = [P1a, P1b]
                    if t == 0:
                        P1l = ppl.tile([128, 512], f32, tag="P1l")
                    for bks in ((0, 1), (2, 3)):
                        for h in (0, 1):
                            for kc in [k for k in ORDER if (k % 4) in bks]:
                                lh = lhs_slice(mts, kc)
                                for j in range(4):
                                    base = N * kc + 512 * j + 256 * h
                                    nc.tensor.matmul(
                                        out=P1h[h][32 * j:32 * (j + 1), :],
                                        lhsT=lh, rhs=s_Wp[:, base:base + 256],
                                        start=(kc == ORDER[0]),
                                        stop=(kc == ORDER[-1]),
                                        tile_position=(0, 32 * j))
                    vh = []
                    for h in (0, 1):
                        if t == 0:
                            for idx, kc in enumerate(ORDER):
                                lh = lhs_slice(mlos, kc)
                                for j in range(4):
                                    base = N * kc + 512 * j + 256 * h
                                    nc.tensor.matmul(
                                        out=P1l[32 * j:32 * (j + 1),
                                                256 * h:256 * (h + 1)],
                                        lhsT=lh, rhs=s_Wp[:, base:base + 256],
                                        start=(idx == 0), stop=(idx == 15),
                                        tile_position=(0, 32 * j))
                            # dt2 += P1l * 2^-25   (one PSUM operand per op)
                            dtl = wp.tile([128, 256], f32, tag=f"dtl{h}")
                            nc.vector.scalar_tensor_tensor(
                                out=dtl[:], in0=P1l[:, 256 * h:256 * (h + 1)],
                                scalar=1.0 / (LOSC * WSCALE), in1=dt2s[h][:],
                                op0=mult, op1=add)
                            dfin = dtl
                        else:
                            dfin = dt2s[h]
                        # tmp = P1 * 2^-14 + dfin ; v = rne(tmp)
                        tmp = wp.tile([128, 256], f32, tag=f"tmp{h}")
                        nc.vector.scalar_tensor_tensor(
                            out=tmp[:], in0=P1h[h][:],
                            scalar=1.0 / WSCALE,
                            in1=dfin[:], op0=mult, op1=add)
                        v = wp.tile([128, 256], f32, tag=f"v{h}")
                        nc.vector.tensor_scalar(
                            out=v[:], in0=tmp[:], scalar1=MAGIC, scalar2=MAGIC,
                            op0=add, op1=sub)
                        vh.append(v)
                    vh_prev = vh

                nc.sync.dma_start(out=d_out[:, 0:256], in_=vh_prev[0][:])
                nc.sync.dma_start(out=d_out[:, 256:512], in_=vh_prev[1][:])

            else:  # legacy fp32 pipelined mode
                vh_prev = None
                for t in range(steps):
                    mts = []
                    for h in (0, 1):
                        if t == 0:
                            src = s_g0[:, 256 * h:256 * (h + 1)]
                            Rt, shift = s_R1, 1.0
                        else:
                            src = vh_prev[h][:]
                            Rt, shift = s_R2, 127.0
                        a = wp.tile([128, 256], f32, tag=f"a{h}")
                        nc.vector.tensor_mul(
                            out=a[:], in0=Rt[:, 256 * h:256 * (h + 1)], in1=src)
                        mneg = wp.tile([128, 256], f32, tag=f"mneg{h}")
                        nc.vector.scalar_tensor_tensor(
                            out=mneg[:], in0=src, scalar=shift, in1=a[:],
                            op0=sub, op1=mult)
                        pT = pp.tile([128, 256], f32, tag=f"pT{h}")
                        for b in range(2):
                            nc.tensor.transpose(
                                pT[:, 128 * b:128 * (b + 1)],
                                mneg[:, 128 * b:128 * (b + 1)], s_id[:])
                        mt = wp.tile([128, 256], f32, tag=f"mTs{h}")
                        nc.scalar.copy(mt[:], pT[:])
                        mts.append(mt)

                    vh = []
                    for h in (0, 1):
                        P1 = pp.tile([128, 256], f32, tag=f"P1h{h}")
                        for idx, kc in enumerate(ORDER):
                            lh = lhs_slice(mts, kc)
                            for j in range(4):
                                base = N * kc + 512 * j + 256 * h
                                nc.tensor.matmul(
                                    out=P1[32 * j:32 * (j + 1), :],
                                    lhsT=lh, rhs=s_Wp[:, base:base + 256],
                                    start=(idx == 0), stop=(idx == 15),
                                    tile_position=(0, 32 * j))
                        tmp = wp.tile([128, 256], f32, tag=f"tmp{h}")
                        nc.vector.tensor_add(
                            out=tmp[:], in0=P1[:],
                            in1=s_C[:, 256 * h:256 * (h + 1)])
                        v = wp.tile([128, 256], f32, tag=f"v{h}")
                        nc.vector.tensor_scalar(
                            out=v[:], in0=tmp[:], scalar1=MAGIC, scalar2=MAGIC,
                            op0=add, op1=sub)
                        vh.append(v)
                    vh_prev = vh

                nc.sync.dma_start(out=d_out[:, 0:256], in_=vh_prev[0][:])
                nc.sync.dma_start(out=d_out[:, 256:512], in_=vh_prev[1][:])

    nc.compile()
    return nc


def _pack_w(Wmat):
    """[N, N] -> [128, 16*N]: column block kc holds rows 128*kc.. of Wmat"""
    return np.ascontiguousarray(
        Wmat.reshape(16, 128, N).transpose(1, 0, 2).reshape(128, 16 * N))


def _host_constants(r, eps, beta, K_local, W_cc, kernel_size, mode):
    """All scale folding in fp64, rounded once at the end."""
    pad = kernel_size // 2
    W64 = W_cc.astype(np.float64)
    C64 = np.zeros((N, N))
    idx = np.arange(N)
    for j in range(kernel_size):
        C64[(idx + j - pad) % N, idx] += np.float64(K_local[j])
    eps64 = eps.astype(np.float64)
    beta64 = beta.astype(np.float64)
    r64 = r.astype(np.float64)
    if mode == "h16":
        # W5neg = -2^14 * (r_n/127) * 0.5*(1-beta_m)*eps_m*(W_cc+C)[n,m]
        W3d = 0.5 * (1 - beta64)[None, :] * eps64[None, :] * (W64 + C64)
        W5neg = (-WSCALE * (r64 / 127.0)[:, None] * W3d).astype(np.float16)
        wp = _pack_w(W5neg)
        ddneg = (-(1 - beta64) * (1 - eps64) * r64 / 127.0).astype(np.float32)
        R1 = ddneg                       # DD vector rides in the R1 slot
        R2 = np.zeros(N, np.float32)     # unused
    else:
        W3 = 0.5 * (1 - beta64)[None, :] * eps64[None, :] * (W64 + C64)
        W3[idx, idx] += (1 - beta64) * (1 - eps64)
        wp = _pack_w((-127.0 * W3).astype(np.float32))
        R1 = r.astype(np.float32)
        R2 = (r64 / (127.0 * 127.0)).astype(np.float32)
    return wp, R1, R2, beta64


def _to_bn(x):
    """[32, 2048] -> scrambled [128, 512]: bn[32*j + b, nt] = x[b, 512*j + nt]"""
    return np.ascontiguousarray(
        x.reshape(BL, 4, 512).transpose(1, 0, 2).reshape(128, 512))


def _from_bn(x):
    return np.ascontiguousarray(
        x.reshape(4, BL, 512).transpose(1, 0, 2).reshape(BL, N))


def _bcast_bn(site):
    """[2048] per-site constant -> scrambled [128, 512] (same for all b)"""
    return np.ascontiguousarray(np.broadcast_to(
        site.reshape(4, 1, 512), (4, BL, 512)).reshape(128, 512))


def kernel(drive, r, eps, beta, K_local, W_cc, steps=64, kernel_size=5, **_kw):
    from concourse.bass_utils import run_bass_kernel_spmd

    drive = np.asarray(drive, dtype=np.float32)
    r = np.asarray(r, dtype=np.float32)
    eps = np.asarray(eps, dtype=np.float32)
    beta = np.asarray(beta, dtype=np.float32)
    K_local = np.asarray(K_local, dtype=np.float32)
    W_cc = np.asarray(W_cc, dtype=np.float32)
    steps = int(steps)
    kernel_size = int(kernel_size)

    lo, hi = np.float32(0.0001), np.float32(1.0 - 0.0001)
    if steps <= 0:
        return np.clip(drive, lo, hi).astype(np.float32)

    wmap, R1, R2, beta64 = _host_constants(
        r, eps, beta, K_local, W_cc, kernel_size, MODE)
    R1_bn = _bcast_bn(R1)
    R2_bn = _bcast_bn(R2)
    ident = np.eye(128, dtype=np.float32)

    key = (steps, MODE)
    if key not in _programs:
        _programs[key] = _build_program(steps, mode=MODE)
    nc = _programs[key]

    in_maps = []
    for c in range(NCORES):
        dslice = drive[BL * c:BL * (c + 1)]
        C127 = (127.0 * beta64[None, :] * dslice.astype(np.float64)).astype(np.float32)
        if MODE == "h16":
            g0 = (np.float32(127.0) * dslice).astype(np.float32)
        else:
            g0 = dslice
        in_maps.append(dict(
            Wp=wmap, g0_bn=_to_bn(g0), C_bn=_to_bn(C127),
            R1_bn=R1_bn, R2_bn=R2_bn, ident=ident))

    global _last_in_maps
    _last_in_maps = in_maps
    res = run_bass_kernel_spmd(nc, in_maps, list(range(NCORES)))

    out = np.empty((B, N), dtype=np.float32)
    for c in range(NCORES):
        v = _from_bn(res.results[c]["v_out"])
        g = (v / np.float32(127.0)).astype(np.float32)
        out[BL * c:BL * (c + 1)] = np.clip(g, lo, hi)
    return out


# revision 22
# speedup vs baseline: 12.1333x; 12.1333x over previous
"""CML int8-sim Trainium2 kernel.

Strategy (data-parallel over batch, 8 cores, B=256 -> 32 rows/core):
The per-step update
    mapped = r*g*(1-g)
    coupled = 0.5*(circ_conv(mapped, K) + mapped @ W_cc)
    g' = (1-beta)*((1-eps)*mapped + eps*coupled) + beta*drive
    g  = round(127*g')/127
is linear in `mapped`, so conv + coupling + site scalings fold into one
matrix.  State is kept 127-scaled: v = 127*g is an EXACT INTEGER in
[0,127] after every round, hence u = v*(127-v) is an even integer
<= 4032 -- exactly representable in fp16.  Folding r into the rows of
the weight matrix and pulling the diagonal passthrough out onto the DVE
makes the whole step a SINGLE fp16 matmul:
    127*g' = u @ W5*2^14 * 2^-14 + dd*u + 127*beta*drive
    W5[n,m] = (r_n/127) * 0.5*(1-beta_m)*eps_m*(W_cc+C)[n,m]   (fp16, 2^14-scaled)
    dd = (1-beta)*(1-eps)*r/127                                 (DVE elementwise)
fp16 streams (vs fp32) also relieve the XBUS budget of the 4-way
col-tiled PE matmuls.  Step 0's state (127*drive) is continuous, so its
lhs uses an exact hi+lo fp16 split (2 matmul chains, one step only).

Layout: scrambled [128, 512] (partition = 32*jgrp + batch) so all
elementwise work uses the full 128 partitions; per-step tensors split
into column halves so the DVE/ACT tail of one half overlaps the PE
matmuls of the other (block-major k order).
"""
import numpy as np

B, N, NCORES = 256, 2048, 8
BL = B // NCORES          # 32 batch rows per core
MAGIC = 12582912.0        # 1.5*2^23: (x+M)-M == RNE round for |x| < 2^22
WSCALE = 16384.0          # 2^14: keeps fp16 W entries out of subnormal range
LOSC = 2048.0             # 2^11: scale for the step-0 lo lhs part

MODE = "h16"              # "h16" (fp16 W, default) | "fp32p" (legacy fp32)
_programs = {}
_last_in_maps = None


def _build_program(steps, mode="h16", wbufs=2):
    import concourse.mybir as mybir
    import concourse.tile as tile
    from concourse import bacc

    f32 = mybir.dt.float32
    f16 = mybir.dt.float16
    sub = mybir.AluOpType.subtract
    add = mybir.AluOpType.add
    mult = mybir.AluOpType.mult

    nc = bacc.Bacc("TRN2", target_bir_lowering=False, debug=False)

    wdt = f16 if mode == "h16" else f32
    d_Wp = nc.dram_tensor("Wp", [128, 16 * N], wdt, kind="ExternalInput").ap()
    d_g0 = nc.dram_tensor("g0_bn", [128, 512], f32, kind="ExternalInput").ap()
    d_C = nc.dram_tensor("C_bn", [128, 512], f32, kind="ExternalInput").ap()
    d_R1 = nc.dram_tensor("R1_bn", [128, 512], f32, kind="ExternalInput").ap()
    d_R2 = nc.dram_tensor("R2_bn", [128, 512], f32, kind="ExternalInput").ap()
    d_id = nc.dram_tensor("ident", [128, 128], f32, kind="ExternalInput").ap()
    d_out = nc.dram_tensor("v_out", [128, 512], f32, kind="ExternalOutput").ap()

    ORDER = [0, 4, 8, 12, 1, 5, 9, 13, 2, 6, 10, 14, 3, 7, 11, 15]

    with tile.TileContext(nc) as tc:
        with tc.tile_pool(name="consts", bufs=1) as cp, \
             tc.tile_pool(name="work", bufs=wbufs) as wp, \
             tc.tile_pool(name="psum", bufs=2, space="PSUM") as pp, \
             tc.tile_pool(name="psumt", bufs=1, space="PSUM") as ppt, \
             tc.tile_pool(name="psuml", bufs=1, space="PSUM") as ppl:
            s_Wp = cp.tile([128, 16 * N], wdt)
            s_g0 = cp.tile([128, 512], f32)
            s_C = cp.tile([128, 512], f32)
            s_R1 = cp.tile([128, 512], f32)
            s_R2 = cp.tile([128, 512], f32)
            s_id = cp.tile([128, 128], f32)
            nc.sync.dma_start(out=s_g0[:], in_=d_g0[:])
            nc.sync.dma_start(out=s_C[:], in_=d_C[:])
            nc.sync.dma_start(out=s_R1[:], in_=d_R1[:])
            nc.sync.dma_start(out=s_R2[:], in_=d_R2[:])
            nc.sync.dma_start(out=s_id[:], in_=d_id[:])
            # W is big: DMA per 128-chunk so first matmuls can start
            # before the whole matrix has landed.
            for kc in range(16):
                nc.sync.dma_start(out=s_Wp[:, N * kc:N * (kc + 1)],
                                  in_=d_Wp[:, N * kc:N * (kc + 1)])

            def lhs_slice(tiles, kc):
                bk = kc % 4
                off = 128 * (bk % 2) + 32 * (kc // 4)
                return tiles[bk // 2][:, off:off + 32]

            if mode == "h16":
                # R1_bn carries DD = -(1-beta)(1-eps)*r/127 broadcast; R2 unused
                s_DD = s_R1
                s_id16 = cp.tile([128, 128], f16)
                nc.vector.tensor_copy(out=s_id16[:], in_=s_id[:])
                vh_prev = None
                for t in range(steps):
                    mts, mlos, dt2s = [], [], []
                    if t == 0:
                        # single bank-wide f32 transpose target (step 0 only)
                        pTm = ppt.tile([128, 512], f32, tag="pT")
                    for h in (0, 1):
                        if t == 0:
                            # continuous state: full-precision path + hi/lo
                            src = s_g0[:, 256 * h:256 * (h + 1)]
                            e1 = wp.tile([128, 256], f32, tag=f"e1{h}")
                            nc.vector.scalar_tensor_tensor(
                                out=e1[:], in0=src, scalar=127.0, in1=src,
                                op0=sub, op1=mult)
                            dsrc = e1
                            for b in range(2):
                                nc.tensor.transpose(
                                    pTm[:, 256 * h + 128 * b:
                                        256 * h + 128 * (b + 1)],
                                    e1[:, 128 * b:128 * (b + 1)], s_id[:])
                            mt = wp.tile([128, 256], f16, tag=f"mT{h}")
                            nc.scalar.copy(mt[:], pTm[:, 256 * h:256 * (h + 1)])
                            mts.append(mt)
                            # lo part: ml = (pT - mt) * 2^11, transposed space
                            rl = wp.tile([128, 256], f32, tag=f"rl{h}")
                            nc.vector.tensor_sub(
                                out=rl[:], in0=pTm[:, 256 * h:256 * (h + 1)],
                                in1=mt[:])
                            ml = wp.tile([128, 256], f16, tag=f"ml{h}")
                            nc.vector.tensor_scalar(
                                out=ml[:], in0=rl[:], scalar1=LOSC,
                                scalar2=None, op0=mult)
                            mlos.append(ml)
                        else:
                            # m16 = (v - 127) * v  == -u : even integer
                            # <= 4032, EXACT in fp16
                            src = vh_prev[h][:]
                            m16 = wp.tile([128, 256], f16, tag=f"m16{h}")
                            nc.vector.scalar_tensor_tensor(
                                out=m16[:], in0=src, scalar=127.0, in1=src,
                                op0=sub, op1=mult)
                            dsrc = m16
                            pT16 = ppt.tile([128, 256], f16, tag=f"pT16{h}")
                            for b in range(2):
                                nc.tensor.transpose(
                                    pT16[:, 128 * b:128 * (b + 1)],
                                    m16[:, 128 * b:128 * (b + 1)], s_id16[:])
                            # split copies so matmuls can start after block 0
                            mt = wp.tile([128, 256], f16, tag=f"mT{h}")
                            nc.scalar.copy(mt[:, 0:128], pT16[:, 0:128])
                            nc.scalar.copy(mt[:, 128:256], pT16[:, 128:256])
                            mts.append(mt)
                        # dt2 = dsrc*DD + C == dd*u + 127*beta*drive (hidden
                        # under the matmuls)
                        dt1 = wp.tile([128, 256], f32, tag=f"dt1{h}")
                        nc.vector.tensor_mul(
                            out=dt1[:], in0=dsrc[:],
                            in1=s_DD[:, 256 * h:256 * (h + 1)])
                        dt2 = wp.tile([128, 256], f32, tag=f"dt2{h}")
                        nc.vector.tensor_add(
                            out=dt2[:], in0=dt1[:],
                            in1=s_C[:, 256 * h:256 * (h + 1)])
                        dt2s.append(dt2)

                    # Separate PSUM bank per half so the two accumulation
                    # chains can interleave at bk-group granularity
                    # [h0|bk01, h1|bk01, h0|bk23, h1|bk23] without two open
                    # groups sharing a bank.  Each chain keeps its internal
                    # round order (numerics unchanged), but the rounds that
                    # need the other half's transposed tile (bk23 ->
                    # mts[1]) land a full group later, giving both
                    # post-round chains ~2 groups of slack before the next
                    # step's matmuls consume their output.
                    P1a = pp.tile([128, 256], f32, tag="P1a")
                    P1b = pp.tile([128, 256], f32, tag="P1b")
                    P1h = [P1a, P1b]
                    if t == 0:
                        P1l = ppl.tile([128, 512], f32, tag="P1l")
                    for bks in ((0, 1), (2, 3)):
                        for h in (0, 1):
                            for kc in [k for k in ORDER if (k % 4) in bks]:
                                lh = lhs_slice(mts, kc)
                                for j in range(4):
                                    base = N * kc + 512 * j + 256 * h
                                    nc.tensor.matmul(
                                        out=P1h[h][32 * j:32 * (j + 1), :],
                                        lhsT=lh, rhs=s_Wp[:, base:base + 256],
                                        start=(kc == ORDER[0]),
                                        stop=(kc == ORDER[-1]),
                                        tile_position=(0, 32 * j))
                    vh = []
                    for h in (0, 1):
                        if t == 0:
                            for idx, kc in enumerate(ORDER):
                                lh = lhs_slice(mlos, kc)
                                for j in range(4):
                                    base = N * kc + 512 * j + 256 * h
                                    nc.tensor.matmul(
                                        out=P1l[32 * j:32 * (j + 1),
                                                256 * h:256 * (h + 1)],
                                        lhsT=lh, rhs=s_Wp[:, base:base + 256],
                                        start=(idx == 0), stop=(idx == 15),
                                        tile_position=(0, 32 * j))
                            # dt2 += P1l * 2^-25   (one PSUM operand per op)
                            dtl = wp.tile([128, 256], f32, tag=f"dtl{h}")
                            nc.vector.scalar_tensor_tensor(
                                out=dtl[:], in0=P1l[:, 256 * h:256 * (h + 1)],
                                scalar=1.0 / (LOSC * WSCALE), in1=dt2s[h][:],
                                op0=mult, op1=add)
                            dfin = dtl
                        else:
                            dfin = dt2s[h]
                        # tmp = P1 * 2^-14 + dfin ; v = rne(tmp)
                        tmp = wp.tile([128, 256], f32, tag=f"tmp{h}")
                        nc.vector.scalar_tensor_tensor(
                            out=tmp[:], in0=P1h[h][:],
                            scalar=1.0 / WSCALE,
                            in1=dfin[:], op0=mult, op1=add)
                        v = wp.tile([128, 256], f32, tag=f"v{h}")
                        nc.vector.tensor_scalar(
                            out=v[:], in0=tmp[:], scalar1=MAGIC, scalar2=MAGIC,
                            op0=add, op1=sub)
                        vh.append(v)
                    vh_prev = vh

                nc.sync.dma_start(out=d_out[:, 0:256], in_=vh_prev[0][:])
                nc.sync.dma_start(out=d_out[:, 256:512], in_=vh_prev[1][:])

            else:  # legacy fp32 pipelined mode
                vh_prev = None
                for t in range(steps):
                    mts = []
                    for h in (0, 1):
                        if t == 0:
                            src = s_g0[:, 256 * h:256 * (h + 1)]
                            Rt, shift = s_R1, 1.0
                        else:
                            src = vh_prev[h][:]
                            Rt, shift = s_R2, 127.0
                        a = wp.tile([128, 256], f32, tag=f"a{h}")
                        nc.vector.tensor_mul(
                            out=a[:], in0=Rt[:, 256 * h:256 * (h + 1)], in1=src)
                        mneg = wp.tile([128, 256], f32, tag=f"mneg{h}")
                        nc.vector.scalar_tensor_tensor(
                            out=mneg[:], in0=src, scalar=shift, in1=a[:],
                            op0=sub, op1=mult)
                        pT = pp.tile([128, 256], f32, tag=f"pT{h}")
                        for b in range(2):
                            nc.tensor.transpose(
                                pT[:, 128 * b:128 * (b + 1)],
                                mneg[:, 128 * b:128 * (b + 1)], s_id[:])
                        mt = wp.tile([128, 256], f32, tag=f"mTs{h}")
                        nc.scalar.copy(mt[:], pT[:])
                        mts.append(mt)

                    vh = []
                    for h in (0, 1):
                        P1 = pp.tile([128, 256], f32, tag=f"P1h{h}")
                        for idx, kc in enumerate(ORDER):
                            lh = lhs_slice(mts, kc)
                            for j in range(4):
                                base = N * kc + 512 * j + 256 * h
                                nc.tensor.matmul(
                                    out=P1[32 * j:32 * (j + 1), :],
                                    lhsT=lh, rhs=s_Wp[:, base:base + 256],
                                    start=(idx == 0), stop=(idx == 15),
                                    tile_position=(0, 32 * j))
                        tmp = wp.tile([128, 256], f32, tag=f"tmp{h}")
                        nc.vector.tensor_add(
                            out=tmp[:], in0=P1[:],
                            in1=s_C[:, 256 * h:256 * (h + 1)])
                        v = wp.tile([128, 256], f32, tag=f"v{h}")
                        nc.vector.tensor_scalar(
                            out=v[:], in0=tmp[:], scalar1=MAGIC, scalar2=MAGIC,
                            op0=add, op1=sub)
                        vh.append(v)
                    vh_prev = vh

                nc.sync.dma_start(out=d_out[:, 0:256], in_=vh_prev[0][:])
                nc.sync.dma_start(out=d_out[:, 256:512], in_=vh_prev[1][:])

    nc.compile()
    return nc


def _pack_w(Wmat):
    """[N, N] -> [128, 16*N]: column block kc holds rows 128*kc.. of Wmat"""
    return np.ascontiguousarray(
        Wmat.reshape(16, 128, N).transpose(1, 0, 2).reshape(128, 16 * N))


def _host_constants(r, eps, beta, K_local, W_cc, kernel_size, mode):
    """All scale folding in fp64, rounded once at the end."""
    pad = kernel_size // 2
    W64 = W_cc.astype(np.float64)
    C64 = np.zeros((N, N))
    idx = np.arange(N)
    for j in range(kernel_size):
        C64[(idx + j - pad) % N, idx] += np.float64(K_local[j])
    eps64 = eps.astype(np.float64)
    beta64 = beta.astype(np.float64)
    r64 = r.astype(np.float64)
    if mode == "h16":
        # W5neg = -2^14 * (r_n/127) * 0.5*(1-beta_m)*eps_m*(W_cc+C)[n,m]
        W3d = 0.5 * (1 - beta64)[None, :] * eps64[None, :] * (W64 + C64)
        W5neg = (-WSCALE * (r64 / 127.0)[:, None] * W3d).astype(np.float16)
        wp = _pack_w(W5neg)
        ddneg = (-(1 - beta64) * (1 - eps64) * r64 / 127.0).astype(np.float32)
        R1 = ddneg                       # DD vector rides in the R1 slot
        R2 = np.zeros(N, np.float32)     # unused
    else:
        W3 = 0.5 * (1 - beta64)[None, :] * eps64[None, :] * (W64 + C64)
        W3[idx, idx] += (1 - beta64) * (1 - eps64)
        wp = _pack_w((-127.0 * W3).astype(np.float32))
        R1 = r.astype(np.float32)
        R2 = (r64 / (127.0 * 127.0)).astype(np.float32)
    return wp, R1, R2, beta64


def _to_bn(x):
    """[32, 2048] -> scrambled [128, 512]: bn[32*j + b, nt] = x[b, 512*j + nt]"""
    return np.ascontiguousarray(
        x.reshape(BL, 4, 512).transpose(1, 0, 2).reshape(128, 512))


def _from_bn(x):
    return np.ascontiguousarray(
        x.reshape(4, BL, 512).transpose(1, 0, 2).reshape(BL, N))


def _bcast_bn(site):
    """[2048] per-site constant -> scrambled [128, 512] (same for all b)"""
    return np.ascontiguousarray(np.broadcast_to(
        site.reshape(4, 1, 512), (4, BL, 512)).reshape(128, 512))


def kernel(drive, r, eps, beta, K_local, W_cc, steps=64, kernel_size=5, **_kw):
    from concourse.bass_utils import run_bass_kernel_spmd

    drive = np.asarray(drive, dtype=np.float32)
    r = np.asarray(r, dtype=np.float32)
    eps = np.asarray(eps, dtype=np.float32)
    beta = np.asarray(beta, dtype=np.float32)
    K_local = np.asarray(K_local, dtype=np.float32)
    W_cc = np.asarray(W_cc, dtype=np.float32)
    steps = int(steps)
    kernel_size = int(kernel_size)

    lo, hi = np.float32(0.0001), np.float32(1.0 - 0.0001)
    if steps <= 0:
        return np.clip(drive, lo, hi).astype(np.float32)

    wmap, R1, R2, beta64 = _host_constants(
        r, eps, beta, K_local, W_cc, kernel_size, MODE)
    R1_bn = _bcast_bn(R1)
    R2_bn = _bcast_bn(R2)
    ident = np.eye(128, dtype=np.float32)

    key = (steps, MODE)
    if key not in _programs:
        _programs[key] = _build_program(steps, mode=MODE)
    nc = _programs[key]

    in_maps = []
    for c in range(NCORES):
        dslice = drive[BL * c:BL * (c + 1)]
        C127 = (127.0 * beta64[None, :] * dslice.astype(np.float64)).astype(np.float32)
        if MODE == "h16":
            g0 = (np.float32(127.0) * dslice).astype(np.float32)
        else:
            g0 = dslice
        in_maps.append(dict(
            Wp=wmap, g0_bn=_to_bn(g0), C_bn=_to_bn(C127),
            R1_bn=R1_bn, R2_bn=R2_bn, ident=ident))

    global _last_in_maps
    _last_in_maps = in_maps
    res = run_bass_kernel_spmd(nc, in_maps, list(range(NCORES)))

    out = np.empty((B, N), dtype=np.float32)
    for c in range(NCORES):
        v = _from_bn(res.results[c]["v_out"])
        g = (v / np.float32(127.0)).astype(np.float32)
        out[BL * c:BL * (c + 1)] = np.clip(g, lo, hi)
    return out
